# revision 1
# baseline (speedup 1.0000x reference)
"""Causal multi-head self-attention with RoPE on 8 Trainium2 NeuronCores.

Sharding (collective-free): 8 cores = 4 batches x 2 query-groups.
Core c handles batch c//2 and query blocks {2i + c%2 : i in 0..7} (interleaved
128-row blocks -> balanced causal work). Each core computes the full K/V
projection for its batch (duplicated within the pair; cheaper than on-chip
collectives), attention for its 1024 queries, and the output projection for
its rows. Host gathers/scatters. One uniform SPMD program; all per-core
asymmetry (query columns, causal masks) is carried in the input data.

On-device layout is transposed ([feature, seq]) so every matmul contracts
over features. RoPE is applied in a de-interleaved x1/x2 layout (full
128-partition DVE ops), then an SBUF->SBUF DMA shuffle regroups Q^T/K^T into
per-head-contiguous chunks for the K=64 attention matmuls (2 heads packed in
the PE array at partition offsets 0/64). Softmax runs in the transposed
layout without max-subtraction (scores ~N(0,1)); the denominator comes from
a ones-column appended to V (M=65 matmul). Projections use fp32r (tf32-like,
full PE rate); attention core and o_proj use bf16 with fp32 PSUM accumulate.
"""

import numpy as np
import ml_dtypes

D = 1024
S = 2048
SQ = 1024          # queries per core
H = 16
DH = 64
NK = 8             # 128-row chunks of D
SCH = 16           # 128-row s-chunks
NJB = 16           # key blocks of 128
THETA = 10000.0
F32 = None         # set lazily (mybir import inside builder)

_cache = {}


def _build():
    import concourse.bass as bass
    import concourse.mybir as mybir
    import concourse.tile as tile
    from concourse import bacc

    f32 = mybir.dt.float32
    f32r = mybir.dt.float32r
    bf16 = mybir.dt.bfloat16

    nc = bacc.Bacc("TRN2")
    xT = nc.dram_tensor("xT", [D, S], f32r, kind="ExternalInput")
    xqT = nc.dram_tensor("xqT", [D, SQ], f32r, kind="ExternalInput")
    wkT = nc.dram_tensor("wkT", [D, D], f32r, kind="ExternalInput")
    wqT = nc.dram_tensor("wqT", [D, D], f32r, kind="ExternalInput")
    wvT = nc.dram_tensor("wvT", [D, D], f32r, kind="ExternalInput")
    woT = nc.dram_tensor("woT", [D, D], bf16, kind="ExternalInput")
    cosk = nc.dram_tensor("cosk", [128, S], bf16, kind="ExternalInput")
    sink = nc.dram_tensor("sink", [128, S], bf16, kind="ExternalInput")
    cosq = nc.dram_tensor("cosq", [128, SQ], bf16, kind="ExternalInput")
    sinq = nc.dram_tensor("sinq", [128, SQ], bf16, kind="ExternalInput")
    masks = nc.dram_tensor("masks", [2, 128, 128], bf16, kind="ExternalInput")
    outT = nc.dram_tensor("outT", [D, SQ], f32, kind="ExternalOutput")

    EXP = mybir.ActivationFunctionType.Exp

    with tile.TileContext(nc) as tc:
        with tc.tile_pool(name="res", bufs=1) as res:
            KT = [res.tile([128, S], bf16, name=f"KT{t}", tag=f"KT{t}")
                  for t in range(NK)]
            QT = [res.tile([128, SQ], bf16, name=f"QT{t}", tag=f"QT{t}")
                  for t in range(NK)]
            V = [res.tile([128, H, DH + 1], bf16, name=f"V{s}", tag=f"V{s}")
                 for s in range(SCH)]
            OT = [res.tile([128, SQ], bf16, name=f"OT{t}", tag=f"OT{t}")
                  for t in range(NK)]
            msk = res.tile([128, 2, 128], bf16)
            nc.sync.dma_start(msk[:, 0, :], masks[0])
            nc.sync.dma_start(msk[:, 1, :], masks[1])

            with tc.tile_pool(name="phA", bufs=1) as phA, \
                 tc.tile_pool(name="psA", bufs=1, space="PSUM") as psA:
                wsb = [phA.tile([128, D], f32r, name=f"w{k}", tag=f"w{k}")
                       for k in range(NK)]
                # ---------- A1: V = x @ wv (x stationary) ----------
                for k in range(NK):
                    nc.gpsimd.dma_start(wsb[k][:], wvT[k * 128:(k + 1) * 128, :])
                for half in range(2):
                    xk = [phA.tile([128, 1024], f32r, name=f"xkA{half}_{k}",
                                   tag=f"xk{k}", bufs=1) for k in range(NK)]
                    for k in range(NK):
                        nc.gpsimd.dma_start(
                            xk[k][:],
                            xT[k * 128:(k + 1) * 128,
                               half * 1024:(half + 1) * 1024])
                    for sc8 in range(8):
                        sc = half * 8 + sc8
                        for dt2 in range(2):
                            pv = psA.tile([128, 512], f32, name=f"pv{sc}_{dt2}",
                                          tag="pv", bufs=2)
                            for k in range(NK):
                                nc.tensor.matmul(
                                    pv[:], xk[k][:, sc8 * 128:(sc8 + 1) * 128],
                                    wsb[k][:, dt2 * 512:(dt2 + 1) * 512],
                                    start=(k == 0), stop=(k == NK - 1))
                            nc.vector.tensor_copy(
                                V[sc][:, dt2 * 8:(dt2 + 1) * 8, 0:DH],
                                pv[:].rearrange("p (h d) -> p h d", h=8))
                        nc.vector.memset(V[sc][:, :, DH:DH + 1], 1.0)

                # ---------- A2/A3: K^T, Q^T projection + rope + shuffle ----
                for which in ("k", "q"):
                    ncols = S if which == "k" else SQ
                    wsrc = wkT if which == "k" else wqT
                    xsrc = xT if which == "k" else xqT
                    csrc = cosk if which == "k" else cosq
                    ssrc = sink if which == "k" else sinq
                    dst = KT if which == "k" else QT
                    cossb = phA.tile([128, ncols], bf16, name=f"cos_{which}",
                                     tag="costab")
                    sinsb = phA.tile([128, ncols], bf16, name=f"sin_{which}",
                                     tag="sintab")
                    nc.gpsimd.dma_start(cossb[:], csrc[:])
                    nc.gpsimd.dma_start(sinsb[:], ssrc[:])
                    for k in range(NK):
                        w2 = phA.tile([128, D], f32r, name=f"w_{which}{k}",
                                      tag=f"w{k}")
                        nc.gpsimd.dma_start(w2[:], wsrc[k * 128:(k + 1) * 128, :])
                        wsb[k] = w2
                    kxf = {}
                    for g in range(4):
                        kxf[g] = (
                            phA.tile([128, ncols], bf16, name=f"kxf1{which}{g}",
                                     tag=f"kxf1_{g}", bufs=1),
                            phA.tile([128, ncols], bf16, name=f"kxf2{which}{g}",
                                     tag=f"kxf2_{g}", bufs=1))
                    for half in range(ncols // 1024):
                        xk2 = [phA.tile([128, 1024], f32r,
                                        name=f"xk{which}{half}_{k}",
                                        tag=f"xk{k}", bufs=1)
                               for k in range(NK)]
                        for k in range(NK):
                            nc.gpsimd.dma_start(
                                xk2[k][:],
                                xsrc[k * 128:(k + 1) * 128,
                                     half * 1024:(half + 1) * 1024])
                        for nt2 in range(2):
                            nt = half * 2 + nt2
                            xrhs = [x[:, nt2 * 512:(nt2 + 1) * 512] for x in xk2]
                            for g in range(4):
                                px1 = psA.tile([128, 512], f32,
                                               name=f"px1_{which}{nt}_{g}",
                                               tag="px1", bufs=2)
                                px2 = psA.tile([128, 512], f32,
                                               name=f"px2_{which}{nt}_{g}",
                                               tag="px2", bufs=2)
                                for k in range(NK):
                                    nc.tensor.matmul(
                                        px1[:], wsb[k][:, (2 * g) * 128:(2 * g + 1) * 128],
                                        xrhs[k][:], start=(k == 0), stop=(k == NK - 1))
                                for k in range(NK):
                                    nc.tensor.matmul(
                                        px2[:], wsb[k][:, (2 * g + 1) * 128:(2 * g + 2) * 128],
                                        xrhs[k][:], start=(k == 0), stop=(k == NK - 1))
                                cs = cossb[:, nt * 512:(nt + 1) * 512]
                                sn = sinsb[:, nt * 512:(nt + 1) * 512]
                                t1 = phA.tile([128, 512], bf16, name=f"t1{which}{nt}{g}",
                                              tag="t1", bufs=2)
                                t2 = phA.tile([128, 512], bf16, name=f"t2{which}{nt}{g}",
                                              tag="t2", bufs=2)
                                kx1, kx2 = kxf[g]
                                c01 = slice(nt * 512, (nt + 1) * 512)
                                nc.vector.tensor_mul(t1[:], px1[:], cs)
                                nc.vector.tensor_mul(t2[:], px2[:], sn)
                                nc.vector.tensor_sub(kx1[:, c01], t1[:], t2[:])
                                nc.vector.tensor_mul(t1[:], px1[:], sn)
                                nc.vector.tensor_mul(t2[:], px2[:], cs)
                                nc.vector.tensor_add(kx2[:, c01], t1[:], t2[:])
                    for g in range(4):
                        kx1, kx2 = kxf[g]
                        for i in range(4):
                            h = 4 * g + i
                            td, rb = h // 2, (h % 2) * 64
                            nc.sync.dma_start(dst[td][rb:rb + 32, :],
                                              kx1[i * 32:(i + 1) * 32, :])
                            nc.sync.dma_start(dst[td][rb + 32:rb + 64, :],
                                              kx2[i * 32:(i + 1) * 32, :])

            # ---------------- B: attention ----------------
            with tc.tile_pool(name="phB", bufs=1) as phB, \
                 tc.tile_pool(name="psS", bufs=1, space="PSUM") as psS, \
                 tc.tile_pool(name="psO", bufs=1, space="PSUM") as psO:
                for t in range(NK):
                    po = [psO.tile([DH + 1, SQ], f32, name=f"po{t}_{i}",
                                   tag=f"po{i}") for i in range(2)]
                    for jb in range(NJB):
                        fl = jb // 2
                        flc = fl * 128
                        pieces = ([(flc, 512), (512, 1024)] if fl < 4
                                  else [(flc, 1024)])
                        for h2 in range(2):
                            hb = h2 * 64
                            ps_s = psS.tile([128, SQ], f32, name=f"s{t}_{jb}_{h2}",
                                            tag=f"s{h2}")
                            for (c0, c1) in pieces:
                                nc.tensor.matmul(
                                    ps_s[:, c0:c1],
                                    KT[t][hb:hb + 64, jb * 128:(jb + 1) * 128],
                                    QT[t][hb:hb + 64, c0:c1],
                                    start=True, stop=True)
                            pt = phB.tile([128, SQ], bf16, name=f"pt{t}_{jb}_{h2}",
                                          tag=f"pt{h2}", bufs=2)
                            nc.scalar.activation(pt[:, flc:SQ], ps_s[:, flc:SQ],
                                                 EXP, scale=0.125)
                            nc.vector.tensor_mul(pt[:, flc:flc + 128],
                                                 pt[:, flc:flc + 128],
                                                 msk[:, jb % 2, :])
                            for (c0, c1) in pieces:
                                nc.tensor.matmul(
                                    po[h2][:, c0:c1],
                                    V[jb][:, 2 * t + h2, :],
                                    pt[:, c0:c1],
                                    start=(jb == 0), stop=(jb == NJB - 1))
                    for h2 in range(2):
                        rr = phB.tile([1, SQ], f32, name=f"rr{t}_{h2}",
                                      tag="rr", bufs=2)
                        nc.vector.reciprocal(rr[:], po[h2][DH:DH + 1, :])
                        bcr = phB.tile([64, SQ], f32, name=f"bcr{t}_{h2}",
                                       tag="bcr", bufs=2)
                        nc.gpsimd.partition_broadcast(bcr[:], rr[:])
                        nc.vector.tensor_mul(OT[t][h2 * 64:(h2 + 1) * 64, :],
                                             po[h2][0:DH, :], bcr[:])

            # ---------------- C: output projection ----------------
            with tc.tile_pool(name="phC", bufs=1) as phC, \
                 tc.tile_pool(name="psC", bufs=1, space="PSUM") as psC:
                wo_sb = [phC.tile([128, D], bf16, name=f"wo{k}", tag=f"wo{k}")
                         for k in range(NK)]
                for k in range(NK):
                    nc.gpsimd.dma_start(wo_sb[k][:], woT[k * 128:(k + 1) * 128, :])
                for et in range(NK):
                    for qt2 in range(2):
                        pp = psC.tile([128, 512], f32, name=f"pp{et}_{qt2}",
                                      tag="pp", bufs=2)
                        for dc in range(NK):
                            nc.tensor.matmul(
                                pp[:], wo_sb[dc][:, et * 128:(et + 1) * 128],
                                OT[dc][:, qt2 * 512:(qt2 + 1) * 512],
                                start=(dc == 0), stop=(dc == NK - 1))
                        so = phC.tile([128, 512], f32, name=f"so{et}_{qt2}",
                                      tag="so", bufs=2)
                        nc.vector.tensor_copy(so[:], pp[:])
                        nc.sync.dma_start(
                            outT[et * 128:(et + 1) * 128,
                                 qt2 * 512:(qt2 + 1) * 512], so[:])
    nc.compile()
    return nc


def _perm_global():
    """perm[g] = original feature index for global (pre-shuffle) row g."""
    perm = np.empty(D, dtype=np.int64)
    for g in range(D):
        c, p = g // 128, g % 128
        group, is_x1 = c // 2, (c % 2 == 0)
        head, freq = 4 * group + p // 32, p % 32
        perm[g] = head * DH + 2 * freq + (0 if is_x1 else 1)
    return perm


def kernel(x, wq, wk, wv, wo, token_positions):
    from concourse.bass_utils import run_bass_kernel_spmd

    if "nc" not in _cache:
        _cache["nc"] = _build()
    nc = _cache["nc"]

    x = np.asarray(x, dtype=np.float32)
    wq = np.asarray(wq, dtype=np.float32)
    wk = np.asarray(wk, dtype=np.float32)
    wv = np.asarray(wv, dtype=np.float32)
    wo = np.asarray(wo, dtype=np.float32)
    pos = np.asarray(token_positions).astype(np.float64)

    inv_freq = THETA ** (-np.arange(0, DH, 2, dtype=np.float64) / DH)  # [32]
    ang = pos[:, None] * inv_freq                                      # [S, 32]
    cos_t = np.cos(ang).T.astype(np.float32)                           # [32, S]
    sin_t = np.sin(ang).T.astype(np.float32)
    cos_full = np.tile(cos_t, (4, 1))                                  # [128, S]
    sin_full = np.tile(sin_t, (4, 1))

    perm = _perm_global()
    wqT_p = np.ascontiguousarray(wq[perm, :].T)
    wkT_p = np.ascontiguousarray(wk[perm, :].T)
    wvT = np.ascontiguousarray(wv.T)
    woT = np.ascontiguousarray(wo.T).astype(ml_dtypes.bfloat16)

    tri = np.triu(np.ones((128, 128), np.float32))   # keep j' <= q'
    m_par = [
        np.stack([tri, np.zeros((128, 128), np.float32)]),   # parity 0
        np.stack([np.ones((128, 128), np.float32), tri]),    # parity 1
    ]

    bf = ml_dtypes.bfloat16
    in_maps = []
    qcols_by_par = []
    for par in range(2):
        qcols_by_par.append(np.concatenate(
            [np.arange((2 * i + par) * 128, (2 * i + par + 1) * 128)
             for i in range(8)]))
    for c in range(8):
        b, par = c // 2, c % 2
        xb = np.ascontiguousarray(x[b].T)             # [D, S]
        qcols = qcols_by_par[par]
        in_maps.append({
            "xT": xb,
            "xqT": np.ascontiguousarray(xb[:, qcols]),
            "wkT": wkT_p, "wqT": wqT_p, "wvT": wvT, "woT": woT,
            "cosk": cos_full.astype(bf), "sink": sin_full.astype(bf),
            "cosq": np.ascontiguousarray(cos_full[:, qcols]).astype(bf),
            "sinq": np.ascontiguousarray(sin_full[:, qcols]).astype(bf),
            "masks": m_par[par].astype(bf),
        })

    _cache["in_maps"] = in_maps
    res = run_bass_kernel_spmd(nc, in_maps, list(range(8)))
    out = np.empty((4, S, D), dtype=np.float32)
    for c in range(8):
        b, par = c // 2, c % 2
        out[b, qcols_by_par[par], :] = res.results[c]["outT"].T
    return out



# revision 5
# speedup vs baseline: 1.2389x; 1.2389x over previous
"""Causal multi-head self-attention with RoPE on 8 Trainium2 NeuronCores.

Sharding (collective-free): 8 cores = 4 batches x 2 query-parity groups.
Core c handles batch c//2 and query blocks {2i + c%2}. Each core computes the
full K/V projection for its batch, attention for its 1024 queries, and the
output projection for its rows. Host gathers/scatters.

All on-device tensors are fp16 (10-bit mantissa ~ tf32 accuracy, full PE
rate, 2x DVE rate); PSUM accumulation is fp32. Phases:
  P1: K-proj (+rope), Q-proj (+rope), V-proj. Weight-stationary matmuls for
      K/Q with x resident in SBUF; rope on DVE+GpSimd; de-interleave shuffle
      via SBUF->SBUF DMA; V-proj x-stationary with PSUM->SBUF copies on the
      (otherwise idle) Scalar engine. Weights prefetch one phase ahead
      through a bufs=2 pool rotation (wk->wq->wv->wo).
  P2: attention, software-pipelined 2-deep around the Scalar engine (exp is
      the bottleneck): per (t, qh, jb): 2 score matmuls (both heads) into one
      [128,2,512] PSUM tile, ONE exp op for both heads, data-driven causal
      mask on DVE, attnV accumulates into double-buffered [65,512] po tiles
      (ones-column denominator). Normalization: reciprocal_approx_fast +
      gpsimd partition_broadcast + DVE mul - all off the PE critical path.
  C:  o_proj, wo prefetched during P2, results DMA'd from PSUM.
"""

import numpy as np
import ml_dtypes

D = 1024
S = 2048
SQ = 1024
H = 16
DH = 64
NK = 8
THETA = 10000.0

_cache = {}


def _build():
    import concourse.bass as bass
    import concourse.mybir as mybir
    import concourse.tile as tile
    from concourse import bacc

    f32 = mybir.dt.float32
    f16 = mybir.dt.float16

    nc = bacc.Bacc("TRN2")
    xT = nc.dram_tensor("xT", [D, S], f16, kind="ExternalInput")
    xqT = nc.dram_tensor("xqT", [D, SQ], f16, kind="ExternalInput")
    wkT = nc.dram_tensor("wkT", [D, D], f16, kind="ExternalInput")
    wqT = nc.dram_tensor("wqT", [D, D], f16, kind="ExternalInput")
    wvT = nc.dram_tensor("wvT", [D, D], f16, kind="ExternalInput")
    woT = nc.dram_tensor("woT", [D, D], f16, kind="ExternalInput")
    cosk = nc.dram_tensor("cosk", [128, S], f16, kind="ExternalInput")
    sink = nc.dram_tensor("sink", [128, S], f16, kind="ExternalInput")
    cosq = nc.dram_tensor("cosq", [128, SQ], f16, kind="ExternalInput")
    sinq = nc.dram_tensor("sinq", [128, SQ], f16, kind="ExternalInput")
    masks = nc.dram_tensor("masks", [2, 2, 128, 128], f16, kind="ExternalInput")
    outT = nc.dram_tensor("outT", [D, SQ], f32, kind="ExternalOutput")

    EXP = mybir.ActivationFunctionType.Exp
    COPY = mybir.ActivationFunctionType.Copy

    with tile.TileContext(nc) as tc:
        with tc.tile_pool(name="res", bufs=1) as res, \
             tc.tile_pool(name="wts", bufs=1) as wts:
            XT = [res.tile([128, S], f16, name=f"XT{k}", tag=f"XT{k}")
                  for k in range(NK)]
            XQ = [res.tile([128, SQ], f16, name=f"XQ{k}", tag=f"XQ{k}")
                  for k in range(NK)]
            KT = [res.tile([128, S], f16, name=f"KT{t}", tag=f"KT{t}")
                  for t in range(NK)]
            QT = [res.tile([128, SQ], f16, name=f"QT{t}", tag=f"QT{t}")
                  for t in range(NK)]
            V = [res.tile([128, H, DH + 1], f16, name=f"V{s}", tag=f"V{s}")
                 for s in range(16)]
            OT = [res.tile([128, SQ], f16, name=f"OT{t}", tag=f"OT{t}")
                  for t in range(NK)]
            ck = res.tile([128, S], f16, name="ck")
            sk = res.tile([128, S], f16, name="sk")
            cq = res.tile([128, SQ], f16, name="cq")
            sq = res.tile([128, SQ], f16, name="sq")
            msk = res.tile([128, 2, 2, 128], f16, name="msk")

            for k in range(NK):
                nc.sync.dma_start(XT[k][:], xT[k * 128:(k + 1) * 128, :])
            for k in range(NK):
                nc.sync.dma_start(XQ[k][:], xqT[k * 128:(k + 1) * 128, :])
            nc.sync.dma_start(ck[:], cosk[:])
            nc.sync.dma_start(sk[:], sink[:])
            nc.sync.dma_start(cq[:], cosq[:])
            nc.sync.dma_start(sq[:], sinq[:])
            for jp in range(2):
                for hd in range(2):
                    nc.sync.dma_start(msk[:, jp, hd, :], masks[jp, hd])

            # ---------------- P1: projections + rope ----------------
            with tc.tile_pool(name="phA", bufs=1) as phA, \
                 tc.tile_pool(name="psA", bufs=1, space="PSUM") as psA:
                # K then Q: weight-stationary, x moving
                for which in ("k", "q"):
                    ncols = S if which == "k" else SQ
                    wsrc = wkT if which == "k" else wqT
                    xsb = XT if which == "k" else XQ
                    cs_t = ck if which == "k" else cq
                    sn_t = sk if which == "k" else sq
                    dst = KT if which == "k" else QT
                    W = [wts.tile([128, D], f16, name=f"w{which}{k}",
                                  tag=f"w{k}", bufs=2) for k in range(NK)]
                    for k in range(NK):
                        nc.sync.dma_start(W[k][:], wsrc[k * 128:(k + 1) * 128, :])
                    for g in range(4):
                        for nt in range(ncols // 512):
                            c01 = slice(nt * 512, (nt + 1) * 512)
                            pxe = psA.tile([128, 512], f32, name=f"pxe{which}{g}{nt}",
                                           tag="pxe", bufs=2)
                            pxo = psA.tile([128, 512], f32, name=f"pxo{which}{g}{nt}",
                                           tag="pxo", bufs=2)
                            for k in range(NK):
                                nc.tensor.matmul(
                                    pxe[:], W[k][:, (2 * g) * 128:(2 * g + 1) * 128],
                                    xsb[k][:, c01], start=(k == 0), stop=(k == NK - 1))
                            for k in range(NK):
                                nc.tensor.matmul(
                                    pxo[:], W[k][:, (2 * g + 1) * 128:(2 * g + 2) * 128],
                                    xsb[k][:, c01], start=(k == 0), stop=(k == NK - 1))
                            cs = cs_t[:, c01]
                            sn = sn_t[:, c01]
                            ce = phA.tile([128, 512], f16, name=f"ce{which}{g}{nt}",
                                          tag="ce", bufs=2)
                            co = phA.tile([128, 512], f16, name=f"co{which}{g}{nt}",
                                          tag="co", bufs=2)
                            t1 = phA.tile([128, 512], f16, name=f"t1{which}{g}{nt}",
                                          tag="t1", bufs=2)
                            t2 = phA.tile([128, 512], f16, name=f"t2{which}{g}{nt}",
                                          tag="t2", bufs=2)
                            t3 = phA.tile([128, 512], f16, name=f"t3{which}{g}{nt}",
                                          tag="t3", bufs=2)
                            t4 = phA.tile([128, 512], f16, name=f"t4{which}{g}{nt}",
                                          tag="t4", bufs=2)
                            kx1 = phA.tile([128, 512], f16, name=f"kx1{which}{g}{nt}",
                                           tag="kx1", bufs=2)
                            kx2 = phA.tile([128, 512], f16, name=f"kx2{which}{g}{nt}",
                                           tag="kx2", bufs=2)
                            nc.scalar.activation(ce[:], pxe[:], COPY)
                            nc.scalar.activation(co[:], pxo[:], COPY)
                            nc.vector.tensor_mul(t1[:], ce[:], cs)
                            nc.gpsimd.tensor_mul(t2[:], co[:], sn)
                            nc.vector.tensor_mul(t3[:], ce[:], sn)
                            nc.gpsimd.tensor_mul(t4[:], co[:], cs)
                            nc.vector.tensor_sub(kx1[:], t1[:], t2[:])
                            nc.vector.tensor_add(kx2[:], t3[:], t4[:])
                            for i in range(4):
                                h = 4 * g + i
                                td, rb = h // 2, (h % 2) * 64
                                nc.sync.dma_start(dst[td][rb:rb + 32, c01],
                                                  kx1[i * 32:(i + 1) * 32, :])
                                nc.sync.dma_start(dst[td][rb + 32:rb + 64, c01],
                                                  kx2[i * 32:(i + 1) * 32, :])

                # V projection: x-stationary, wv moving; copies on Scalar
                WV = [wts.tile([128, D], f16, name=f"wv{k}", tag=f"w{k}",
                               bufs=2) for k in range(NK)]
                for k in range(NK):
                    nc.sync.dma_start(WV[k][:], wvT[k * 128:(k + 1) * 128, :])
                for dt2 in range(2):
                    for sc in range(16):
                        pv = psA.tile([128, 512], f32, name=f"pv{dt2}{sc}",
                                      tag="pv", bufs=2)
                        for k in range(NK):
                            nc.tensor.matmul(
                                pv[:], XT[k][:, sc * 128:(sc + 1) * 128],
                                WV[k][:, dt2 * 512:(dt2 + 1) * 512],
                                start=(k == 0), stop=(k == NK - 1))
                        nc.scalar.activation(
                            V[sc][:, dt2 * 8:(dt2 + 1) * 8, 0:DH],
                            pv[:].rearrange("p (h d) -> p h d", h=8), COPY)
                        if dt2 == 0:
                            nc.gpsimd.memset(V[sc][:, :, DH:DH + 1], 1.0)

            # prefetch wo during P2
            WO = [wts.tile([128, D], f16, name=f"wo{k}", tag=f"w{k}", bufs=2)
                  for k in range(NK)]
            for k in range(NK):
                nc.sync.dma_start(WO[k][:], woT[k * 128:(k + 1) * 128, :])

            # ---------------- P2: attention ----------------
            with tc.tile_pool(name="phB", bufs=1) as phB, \
                 tc.tile_pool(name="psB", bufs=1, space="PSUM") as psB:
                for t in range(NK):
                    for qh in range(2):
                        jmax = 8 if qh == 0 else 16
                        q01 = slice(qh * 512, (qh + 1) * 512)
                        po = [psB.tile([DH + 1, 512], f32, name=f"po{t}{qh}{i}",
                                       tag=f"po{i}", bufs=2) for i in range(2)]
                        pend = None  # (jb, pts, flc)
                        for jb in range(jmax):
                            fnb = max(0, jb // 2 - 4 * qh)
                            flc = fnb * 128
                            TT = psB.tile([128, 2, 512], f32, name=f"TT{t}{qh}{jb}",
                                          tag="TT", bufs=2)
                            for h2 in range(2):
                                hb = h2 * 64
                                nc.tensor.matmul(
                                    TT[:, h2, flc:512],
                                    KT[t][hb:hb + 64, jb * 128:(jb + 1) * 128],
                                    QT[t][hb:hb + 64, qh * 512 + flc:(qh + 1) * 512],
                                    start=True, stop=True)
                            pts = phB.tile([128, 2, 512], f16, name=f"pt{t}{qh}{jb}",
                                           tag="pt", bufs=3)
                            nc.scalar.activation(pts[:, :, flc:512], TT[:, :, flc:512],
                                                 EXP, scale=0.125)
                            if jb >= 8 * qh:
                                db = jb // 2 - 4 * qh
                                dbs = slice(db * 128, (db + 1) * 128)
                                nc.vector.tensor_mul(pts[:, :, dbs], pts[:, :, dbs],
                                                     msk[:, jb % 2, :, :])
                            # emit previous jb's attnV now (1-deep lookahead)
                            if pend is not None:
                                pj, ppts, pflc = pend
                                for h2 in range(2):
                                    nc.tensor.matmul(
                                        po[h2][:, pflc:512],
                                        V[pj][:, 2 * t + h2, :],
                                        ppts[:, h2, pflc:512],
                                        start=(pj == 0), stop=False)
                            pend = (jb, pts, flc)
                        pj, ppts, pflc = pend
                        for h2 in range(2):
                            nc.tensor.matmul(
                                po[h2][:, pflc:512], V[pj][:, 2 * t + h2, :],
                                ppts[:, h2, pflc:512],
                                start=(pj == 0), stop=True)
                        for h2 in range(2):
                            rr = phB.tile([1, 512], f32, name=f"rr{t}{qh}{h2}",
                                          tag="rr", bufs=2)
                            nc.vector.reciprocal(rr[:], po[h2][DH:DH + 1, :])
                            bc = phB.tile([64, 512], f32, name=f"bc{t}{qh}{h2}",
                                          tag="bc", bufs=2)
                            nc.gpsimd.partition_broadcast(bc[:], rr[:])
                            nc.vector.tensor_mul(OT[t][h2 * 64:(h2 + 1) * 64, q01],
                                                 po[h2][0:DH, :], bc[:])

            # ---------------- C: output projection ----------------
            with tc.tile_pool(name="phC", bufs=1) as phC, \
                 tc.tile_pool(name="psC", bufs=1, space="PSUM") as psC:
                for et in range(NK):
                    for qt2 in range(2):
                        pp = psC.tile([128, 512], f32, name=f"pp{et}{qt2}",
                                      tag="pp", bufs=2)
                        for dc in range(NK):
                            nc.tensor.matmul(
                                pp[:], WO[dc][:, et * 128:(et + 1) * 128],
                                OT[dc][:, qt2 * 512:(qt2 + 1) * 512],
                                start=(dc == 0), stop=(dc == NK - 1))
                        so = phC.tile([128, 512], f32, name=f"so{et}{qt2}",
                                      tag="so", bufs=2)
                        nc.scalar.activation(so[:], pp[:], COPY)
                        nc.sync.dma_start(
                            outT[et * 128:(et + 1) * 128,
                                 qt2 * 512:(qt2 + 1) * 512], so[:])
    nc.compile()
    return nc


def _perm_global():
    """perm[g] = original feature index for global (pre-shuffle) row g."""
    perm = np.empty(D, dtype=np.int64)
    for g in range(D):
        c, p = g // 128, g % 128
        group, is_x1 = c // 2, (c % 2 == 0)
        head, freq = 4 * group + p // 32, p % 32
        perm[g] = head * DH + 2 * freq + (0 if is_x1 else 1)
    return perm


def kernel(x, wq, wk, wv, wo, token_positions):
    from concourse.bass_utils import run_bass_kernel_spmd

    if "nc" not in _cache:
        _cache["nc"] = _build()
    nc = _cache["nc"]

    x = np.asarray(x, dtype=np.float32)
    wq = np.asarray(wq, dtype=np.float32)
    wk = np.asarray(wk, dtype=np.float32)
    wv = np.asarray(wv, dtype=np.float32)
    wo = np.asarray(wo, dtype=np.float32)
    pos = np.asarray(token_positions).astype(np.float64)

    inv_freq = THETA ** (-np.arange(0, DH, 2, dtype=np.float64) / DH)  # [32]
    ang = pos[:, None] * inv_freq                                      # [S, 32]
    cos_t = np.cos(ang).T.astype(np.float32)                           # [32, S]
    sin_t = np.sin(ang).T.astype(np.float32)
    cos_full = np.tile(cos_t, (4, 1))                                  # [128, S]
    sin_full = np.tile(sin_t, (4, 1))

    fp16 = np.float16
    perm = _perm_global()
    wqT_p = np.ascontiguousarray(wq[perm, :].T).astype(fp16)
    wkT_p = np.ascontiguousarray(wk[perm, :].T).astype(fp16)
    wvT = np.ascontiguousarray(wv.T).astype(fp16)
    woT = np.ascontiguousarray(wo.T).astype(fp16)

    tri = np.triu(np.ones((128, 128), np.float32))   # keep j' <= q'
    m_par = [
        np.stack([tri, np.zeros((128, 128), np.float32)]),   # parity 0
        np.stack([np.ones((128, 128), np.float32), tri]),    # parity 1
    ]

    in_maps = []
    qcols_by_par = []
    for par in range(2):
        qcols_by_par.append(np.concatenate(
            [np.arange((2 * i + par) * 128, (2 * i + par + 1) * 128)
             for i in range(8)]))
    for c in range(8):
        b, par = c // 2, c % 2
        xb = np.ascontiguousarray(x[b].T).astype(fp16)       # [D, S]
        qcols = qcols_by_par[par]
        # masks[jb%2, h2dup]: duplicate along h2
        m2 = np.stack([m_par[par][0], m_par[par][0],
                       m_par[par][1], m_par[par][1]]).reshape(2, 2, 128, 128)
        m2 = np.stack([np.stack([m_par[par][0]] * 2),
                       np.stack([m_par[par][1]] * 2)])        # [2,2,128,128]
        in_maps.append({
            "xT": xb,
            "xqT": np.ascontiguousarray(xb[:, qcols]),
            "wkT": wkT_p, "wqT": wqT_p, "wvT": wvT, "woT": woT,
            "cosk": cos_full.astype(fp16), "sink": sin_full.astype(fp16),
            "cosq": np.ascontiguousarray(cos_full[:, qcols]).astype(fp16),
            "sinq": np.ascontiguousarray(sin_full[:, qcols]).astype(fp16),
            "masks": m2.astype(fp16),
        })

    _cache["in_maps"] = in_maps
    res = run_bass_kernel_spmd(nc, in_maps, list(range(8)))
    out = np.empty((4, S, D), dtype=np.float32)
    for c in range(8):
        b, par = c // 2, c % 2
        out[b, qcols_by_par[par], :] = res.results[c]["outT"].T
    return out
